# revision 16
# baseline (speedup 1.0000x reference)
"""MAM dense kernel for Trainium2 (8 NeuronCores, SPMD data-parallel over M).

C[m,n] = max_k(x[m,k]*w[n,k]) + min_k(x[m,k]*w[n,k]) + bias[n]

Strategy per core (M_c = 512 rows of x), k-on-partitions layout:
  - Host pre-transposes x and w; the kernel loads
      wt  [k_p=128, kt=8, n=1024]  (w transposed, f32)
      xt  [k_p=128, kt=8, m=512]   (x transposed, f32)
    so x[m, kt*128+k_p] is a per-partition scalar.
  - Rows are processed in pairs (J=2) to halve DVE instruction overhead:
      Act:  products q2[k_p, j, kt, n] = wt * x_scalar via activation(Copy,
            scale=xt[:, kt, m+j]) -- 16 instructions per pair (f32 -> f16)
      DVE:  kt-fold tree rounds 8->4->2->1 on both rows at once (f16 TT,
            2x mode, 3 instructions per op per pair)
      DMA:  XBAR transpose lands the [128, 2048] max/min partials
            n-on-partition in SBUF fp16 (out[p,i,:] = in[:,i*128+p])
      DVE:  folds k_p 128->1 per op with 2x TT rounds + a reduce tail
            into per-block collectors
  - Per 128-row block: combine max+min+bias (f32), DMA out as
    o[n_p, nb, m]; the host transposes back to [m, n] (cheap numpy).
"""

import os
import sys

sys.path.insert(0, "/opt/trn_rl_repo")

import numpy as np

M, K, N = 4096, 1024, 1024
N_CORES = 8
M_C = M // N_CORES  # 512 rows per core
KT = K // 128  # 8 k-tiles
NB = N // 128  # 8 n-tiles

_last_results = None  # BassKernelResults from the most recent run (for test.py)


def _build_nc():
    import concourse.bacc as bacc
    import concourse.mybir as mybir
    import concourse.tile as tile
    from contextlib import ExitStack

    f32 = mybir.dt.float32
    f16 = mybir.dt.float16
    amax = mybir.AluOpType.max
    amin = mybir.AluOpType.min
    aadd = mybir.AluOpType.add
    CopyF = mybir.ActivationFunctionType.Copy

    n_blocks = M_C // 128

    nc = bacc.Bacc("TRN2", target_bir_lowering=False, debug=False)
    wt_d = nc.dram_tensor("wt", [K, N], f32, kind="ExternalInput").ap()
    xt_d = nc.dram_tensor("xt", [K, M_C], f32, kind="ExternalInput").ap()
    b_d = nc.dram_tensor("b", [N], f32, kind="ExternalInput").ap()
    # output in (n_p, nb, m) layout; host transposes back to [m, n]
    o_d = nc.dram_tensor("o", [128, NB, M_C], f32, kind="ExternalOutput").ap()

    with tile.TileContext(nc) as tc, ExitStack() as ctx:
        p_const = ctx.enter_context(tc.tile_pool(name="const", bufs=1))

        # --- preamble: load wt, xt, bias (n-layout)
        wt32 = p_const.tile([128, KT, N], f32)
        xt_sb = p_const.tile([128, KT, M_C], f32)
        bias_t = p_const.tile([128, NB], f32)
        nc.sync.dma_start(wt32[:], wt_d.rearrange("(kt p) n -> p kt n", p=128))
        nc.sync.dma_start(xt_sb[:], xt_d.rearrange("(kt p) m -> p kt m", p=128))
        nc.sync.dma_start(bias_t[:], b_d.rearrange("(nb p) -> p nb", p=128))

        p_q = ctx.enter_context(tc.tile_pool(name="q", bufs=2))
        p_t1 = ctx.enter_context(tc.tile_pool(name="t1", bufs=1))
        p_t3 = ctx.enter_context(tc.tile_pool(name="t3", bufs=3))
        p_acc = ctx.enter_context(tc.tile_pool(name="acc", bufs=2))
        p_out = ctx.enter_context(tc.tile_pool(name="out", bufs=1))
        p_tt = ctx.enter_context(tc.tile_pool(name="tt", bufs=3))
        p_pf = ctx.enter_context(tc.tile_pool(name="pf", bufs=1))

        H = KT // 2 * N  # flat half size (4096)

        for b in range(n_blocks):
            # block collectors: column mm holds row m's [n_p, nb] results
            mxc = p_acc.tile([128, NB, 128], f32, tag="mxc")
            mnc = p_acc.tile([128, NB, 128], f32, tag="mnc")
            for mm in range(0, 128, 2):
                m = b * 128 + mm
                q2 = p_q.tile([128, 2, KT, N], f16, tag="q2")
                for j in range(2):
                    for kt in range(KT):
                        nc.scalar.activation(
                            q2[:, j, kt],
                            wt32[:, kt],
                            CopyF,
                            scale=xt_sb[:, kt, m + j : m + j + 1],
                        )
                q2f = q2[:].rearrange("p j kt n -> p j (kt n)")
                b1 = p_t1.tile([128, 2, H], f16, tag="b1")
                a3 = p_t3.tile([128, 2, N], f16, tag="a3")
                b3 = p_t3.tile([128, 2, N], f16, tag="b3")
                # kt-fold rounds 8->4->2->1, both rows per instruction.
                # The max tree folds IN PLACE into q2's own space (q2's
                # first half is dead once both round-1 ops have read it),
                # so the min round-1 must be emitted first.
                a1v = q2f[:, :, 0:H]
                a2v = q2f[:, :, 0 : 2 * N]
                b2v = q2f[:, :, 2 * N : 4 * N]
                nc.vector.tensor_tensor(
                    b1[:], q2f[:, :, 0:H], q2f[:, :, H : 2 * H], amin
                )
                nc.vector.tensor_tensor(
                    a1v, q2f[:, :, 0:H], q2f[:, :, H : 2 * H], amax
                )
                nc.vector.tensor_tensor(
                    a2v, a1v[:, :, 0 : 2 * N], a1v[:, :, 2 * N : 4 * N], amax
                )
                nc.vector.tensor_tensor(
                    b2v, b1[:, :, 0 : 2 * N], b1[:, :, 2 * N : 4 * N], amin
                )
                nc.vector.tensor_tensor(a3[:], a2v[:, :, 0:N], a2v[:, :, N : 2 * N], amax)
                nc.vector.tensor_tensor(b3[:], b2v[:, :, 0:N], b2v[:, :, N : 2 * N], amin)
                # XBAR DMA transpose: partials land n-on-partition in SBUF
                # f16 (out[p, i, :] = in[:, i*128+p]; i = j*NB + nb)
                ta = p_tt.tile([128, 2 * NB, 128], f16, tag="ta")
                tb = p_tt.tile([128, 2 * NB, 128], f16, tag="tb")
                nc.sync.dma_start_transpose(ta[:], a3[:].rearrange("p j n -> p (j n)"))
                nc.sync.dma_start_transpose(tb[:], b3[:].rearrange("p j n -> p (j n)"))
                # DVE: fold k_p 128->1 with 2x TT rounds + small reduce tail
                f4 = p_pf.tile([128, 2 * NB, 64], f16, tag="f4")
                g4 = p_pf.tile([128, 2 * NB, 64], f16, tag="g4")
                nc.vector.tensor_tensor(f4[:], ta[:, :, 0:64], ta[:, :, 64:128], amax)
                nc.vector.tensor_tensor(
                    f4[:, :, 0:32], f4[:, :, 0:32], f4[:, :, 32:64], amax
                )
                nc.vector.tensor_reduce(
                    mxc[:, :, mm : mm + 2].rearrange("p nb j -> p j nb"),
                    f4[:].rearrange("p (j nb) f -> p j nb f", j=2)[:, :, :, 0:32],
                    axis=mybir.AxisListType.X,
                    op=amax,
                )
                nc.vector.tensor_tensor(g4[:], tb[:, :, 0:64], tb[:, :, 64:128], amin)
                nc.vector.tensor_tensor(
                    g4[:, :, 0:32], g4[:, :, 0:32], g4[:, :, 32:64], amin
                )
                nc.vector.tensor_reduce(
                    mnc[:, :, mm : mm + 2].rearrange("p nb j -> p j nb"),
                    g4[:].rearrange("p (j nb) f -> p j nb f", j=2)[:, :, :, 0:32],
                    axis=mybir.AxisListType.X,
                    op=amin,
                )
            # block epilogue on DVE: combine + bias, DMA out
            out_sb = p_out.tile([128, NB, 128], f32, tag="out")
            nc.vector.tensor_tensor(out_sb[:], mxc[:], mnc[:], aadd)
            nc.vector.tensor_tensor(
                out_sb[:],
                out_sb[:],
                bias_t[:].unsqueeze(2).broadcast_to([128, NB, 128]),
                aadd,
            )
            nc.sync.dma_start(o_d[:, :, b * 128 : (b + 1) * 128], out_sb[:])

    nc.compile()
    return nc


def kernel(x: np.ndarray, weight: np.ndarray, bias: np.ndarray) -> np.ndarray:
    global _last_results
    from concourse.bass_utils import run_bass_kernel_spmd

    try:  # NTFF tracing needs antenv.axon_hooks; disable if unavailable
        import antenv.axon_hooks  # noqa: F401
    except ImportError:
        os.environ["BASS_NEVER_TRACE"] = "1"

    x = np.ascontiguousarray(x, dtype=np.float32)
    weight = np.ascontiguousarray(weight, dtype=np.float32)
    bias = np.ascontiguousarray(bias, dtype=np.float32)

    wt = np.ascontiguousarray(weight.T)  # [K, N]

    nc = _build_nc()
    core_ids = list(range(N_CORES))
    in_maps = [
        {
            "wt": wt,
            "xt": np.ascontiguousarray(x[c * M_C : (c + 1) * M_C].T),  # [K, M_C]
            "b": bias,
        }
        for c in core_ids
    ]
    res = run_bass_kernel_spmd(nc, in_maps, core_ids)
    _last_results = res

    out = np.empty((M, N), dtype=np.float32)
    for c in core_ids:
        # o[n_p, nb, m] -> out[m, nb*128 + n_p]
        o_alt = res.results[c]["o"]
        out[c * M_C : (c + 1) * M_C, :] = o_alt.transpose(2, 1, 0).reshape(M_C, N)
    return out
